# revision 10
# baseline (speedup 1.0000x reference)
"""Trainium2 Bass kernel for LDM-style cross-attention fusion.

Problem (hardcoded shapes):
  x:       [8, 3136, 64]   queries source
  context: [8, 3136, 64]   key/value source
  Wq/Wk/Wv/Wo: [64, 64], bo: [64]
  2 heads x 32 dim, softmax over full 3136x3136 attention matrix.

Sharding: pure data parallel over the batch axis (8 batches -> 8 cores).
Each core computes one batch element entirely on-chip (flash-style: the
[2, 3136, 3136] attention matrix never touches HBM).

Host-side marshalling (pure data movement, no math): x/ctx are sliced per
batch and transposed to channel-major [64, n] (ctx zero-padded to 3200),
so every device DMA moves long contiguous runs (64 descriptors instead of
3072); the kernel returns Z^T [64, 3136] per core and the host transposes
back while unsharding.

Per-core dataflow:
  phase 1: QT = (Wq*scale)^T @ xT, KT = Wk^T @ cT   [64, n] each
           V = ctx @ Wv stored as V_aug [3200, 66] with a ones column per
           head (the PV matmul then also emits the softmax denominator).
  phase 2: for each q-block (448), units u = (k-chunk, head):
              S^T[k,q] = K @ Q^T     (K=32 matmuls, heads in separate PE
                                      row strips via base_partition)
              P^T = exp(S^T)         (ACT, batched over GRP units)
              PV[33, q] += V_aug^T @ P^T   (row 32 = denominator; h1 on
                  psum partitions 64.. -> disjoint PE column strips)
  phase 3: rdsum = 1 / dsum  (dsum pre-memset to 1.0 so untouched rows
           stay finite).
  phase 4: per 128-q-chunk: bcast = ind33^T @ rdsum (outer-product
           broadcast of 1/denom onto the 64 O^T partitions), O^T_n = O^T *
           bcast, Z^T = Wo^T @ O^T_n, + bias via per-partition scalar add.
"""

from contextlib import ExitStack, nullcontext

import numpy as np

import concourse.bass as bass
import concourse.mybir as mybir
import concourse.tile as tile
from concourse import bacc, bass_utils

F32 = mybir.dt.float32
BF16 = mybir.dt.bfloat16
AF = mybir.ActivationFunctionType

N = 3136          # query tokens
NK = 3136         # context tokens
NKP = 3200        # context tokens padded to 25*128
C = 64            # channels
H = 2             # heads
DH = 32           # head dim
INNER = H * DH    # 64
QB = 448          # q block size (free dim of S^T matmuls; fits psum bank)
NQB = N // QB     # 7
KC = 128          # k chunk (partition dim of S^T tiles)
NKC = NKP // KC   # 25
VW = DH + 1       # V columns per head incl. ones column (33)
SCALE = float(DH) ** -0.5
GRP = 3           # exp batch: units per S psum tile (3 banks)

N_CORES = 8


def _ts(i, s):
    return slice(i * s, (i + 1) * s)


def build_kernel(n_cores=N_CORES, repeat=None, grp=GRP, pv_bufs=1, s_bufs=2):
    rep_phase, rep_n = (None, None)
    if isinstance(repeat, tuple):
        rep_phase, rep_n = repeat
    elif repeat:
        rep_phase, rep_n = "all", repeat

    nc = bacc.Bacc(
        "TRN2",
        target_bir_lowering=False,
        debug=False,
        enable_asserts=False,
        num_devices=n_cores,
    )
    xT_d = nc.dram_tensor("xT", [C, N], BF16, kind="ExternalInput").ap()
    cT_d = nc.dram_tensor("cT", [C, NKP], BF16, kind="ExternalInput").ap()
    wq_d = nc.dram_tensor("wq", [C, INNER], BF16, kind="ExternalInput").ap()
    wk_d = nc.dram_tensor("wk", [C, INNER], BF16, kind="ExternalInput").ap()
    wv_d = nc.dram_tensor("wv", [C, INNER], BF16, kind="ExternalInput").ap()
    wo_d = nc.dram_tensor("wo", [INNER, C], BF16, kind="ExternalInput").ap()
    bo_d = nc.dram_tensor("bo", [C, 1], F32, kind="ExternalInput").ap()
    y_d = nc.dram_tensor("y", [C, N], F32, kind="ExternalOutput").ap()

    with tile.TileContext(nc) as tc, ExitStack() as ctx:

        def phase_ctx(name):
            return tc.For_i(0, rep_n, 1) if rep_phase == name else nullcontext()

        if rep_phase == "all":
            ctx.enter_context(tc.For_i(0, rep_n, 1))
        persist = ctx.enter_context(tc.tile_pool(name="persist", bufs=1))

        # ---- constants + inputs ----
        wq = persist.tile([C, INNER], BF16)
        nc.sync.dma_start(wq[:], wq_d[:])
        wk = persist.tile([C, INNER], BF16)
        nc.sync.dma_start(wk[:], wk_d[:])
        wv = persist.tile([C, INNER], BF16)
        nc.sync.dma_start(wv[:], wv_d[:])
        wo = persist.tile([INNER, C], BF16)
        nc.sync.dma_start(wo[:], wo_d[:])
        boT = persist.tile([C, 1], F32)
        nc.sync.dma_start(boT[:], bo_d[:])
        xT = persist.tile([C, N], BF16)
        nc.sync.dma_start(xT[:], xT_d[:])
        cT = persist.tile([C, NKP], BF16)
        nc.sync.dma_start(cT[:], cT_d[:])

        # indicator [33, 64]: row 0 -> head-0 cols, row 32 -> head-1 cols
        ind33 = persist.tile([33, C], F32)
        nc.vector.memset(ind33[:], 0.0)
        nc.vector.memset(ind33[0:1, 0:DH], 1.0)
        nc.vector.memset(ind33[32:33, DH : 2 * DH], 1.0)

        # ================= phase 1: projections =================
        qT = persist.tile([INNER, N], BF16)    # Q^T (pre-scaled via host wq)
        kT = persist.tile([INNER, NKP], BF16)  # K^T
        vaug = persist.tile([128, NKC * 2 * VW], BF16)  # [h0 32|1|h1 32|1] per chunk
        with tc.tile_pool(name="ps_proj", bufs=3, space="PSUM") as ps_proj, \
             tc.tile_pool(name="ps_v", bufs=2, space="PSUM") as ps_v, \
             phase_ctx("p1"):
            # interleave Q^T (copies on DVE) and K^T (copies on ACT) blocks so
            # both copy engines drain the proj psums concurrently
            for qb in range(NQB + 1):
                if qb < NQB:
                    pp = ps_proj.tile([INNER, QB], F32, tag="pq")
                    nc.tensor.matmul(
                        pp[:], wq[:], xT[:, _ts(qb, QB)], start=True, stop=True
                    )
                    nc.vector.tensor_copy(qT[:, _ts(qb, QB)], pp[:])
                # K^T over padded width (cT pad columns are zero from host)
                w = min(QB, NKP - qb * QB)
                if w > 0:
                    pp = ps_proj.tile([INNER, QB], F32, tag="pk")
                    nc.tensor.matmul(
                        pp[:, 0:w], wk[:], cT[:, qb * QB : qb * QB + w],
                        start=True, stop=True,
                    )
                    nc.scalar.copy(kT[:, qb * QB : qb * QB + w], pp[:, 0:w])
            # V chunks -> vaug (data cols; ones cols via memset)
            for k in range(NKC):
                pv = ps_v.tile([128, INNER], F32, tag="pv")
                nc.tensor.matmul(pv[:], cT[:, _ts(k, 128)], wv[:], start=True, stop=True)
                base = k * 2 * VW
                dst_ap = vaug[:, base : base + 2 * VW].rearrange(
                    "p (h w) -> p h w", h=2
                )[:, :, 0:DH]
                src_ap = pv[:].rearrange("p (h w) -> p h w", h=2)
                nc.vector.tensor_copy(dst_ap, src_ap)
            ones_ap = vaug[:].rearrange("p (k h w) -> p k h w", k=NKC, h=2)[
                :, 0 : NKC - 1, :, DH : DH + 1
            ]
            nc.vector.memset(ones_ap, 1.0)
            last = vaug[:, (NKC - 1) * 2 * VW :].rearrange("p (h w) -> p h w", h=2)
            nc.vector.memset(last[0:64, :, DH : DH + 1], 1.0)
            nc.vector.memset(last[64:128, :, DH : DH + 1], 0.0)

        # ================= phase 2: attention =================
        # q blocks of 512 (exactly one psum bank) with a ragged 64 tail
        QBLKS = [(i * 512, 512) for i in range(N // 512)]
        if N % 512:
            QBLKS.append(((N // 512) * 512, N % 512))
        oT = persist.tile([INNER, N], F32)    # unnormalized O^T (both heads)
        dsum = persist.tile([33, N], F32)     # rows 0 / 32: denom per head
        rdsum = persist.tile([33, N], F32)    # 1/denom, filled per q-block
        rdsb = persist.tile([33, N], BF16)    # bf16 copy for the bcast matmul
        nc.vector.memset(dsum[:], 1.0)        # keep untouched rows finite
        NU = 2 * NKC  # 50 units per q block
        with tc.tile_pool(name="ps_s", bufs=s_bufs, space="PSUM") as ps_s, \
             tc.tile_pool(name="ps_pv0", bufs=pv_bufs, space="PSUM") as ps_pv0, \
             tc.tile_pool(name="ps_pv1", bufs=pv_bufs, space="PSUM") as ps_pv1, \
             tc.tile_pool(name="pt", bufs=3) as ptp, \
             phase_ctx("p2"):
            for qo, qw in QBLKS:
                qsl = slice(qo, qo + qw)
                pv0 = ps_pv0.tile([VW, 512], F32, tag="pvacc0")
                pv1 = ps_pv1.tile([64 + VW, 512], F32, tag="pvacc1")
                pvout = [pv0[0:VW, 0:qw], pv1[64 : 64 + VW, 0:qw]]

                def emit_s_exp(g):
                    units = [g * grp + j for j in range(grp) if g * grp + j < NU]
                    nu = len(units)
                    s = ps_s.tile([128, grp * 512], F32, tag="s")
                    for j, u in enumerate(units):
                        k, h = divmod(u, 2)
                        nc.tensor.matmul(
                            s[:, j * 512 : j * 512 + qw],
                            kT[_ts(h, DH), _ts(k, 128)],
                            qT[_ts(h, DH), qsl],
                            start=True, stop=True,
                        )
                    p = ptp.tile([128, grp * 512], BF16, tag="p")
                    nc.scalar.activation(
                        p[:].rearrange("q (j w) -> q j w", w=512)[:, 0:nu, 0:qw],
                        s[:].rearrange("q (j w) -> q j w", w=512)[:, 0:nu, 0:qw],
                        AF.Exp,
                    )
                    return units, p

                def emit_pv(units, p):
                    for j, u in enumerate(units):
                        k, h = divmod(u, 2)
                        base = k * 2 * VW + h * VW
                        nc.tensor.matmul(
                            pvout[h],
                            vaug[:, base : base + VW],
                            p[:, j * 512 : j * 512 + qw],
                            start=(k == 0), stop=(k == NKC - 1),
                        )

                # software pipeline: keep S(g+1) ahead of PV(g) in the PE
                # stream so the PE streams S while ACT runs exp(g), instead
                # of stalling in-order on the exp -> PV dependency.
                ngr = (NU + grp - 1) // grp
                prev = emit_s_exp(0)
                for g in range(1, ngr):
                    cur = emit_s_exp(g)
                    emit_pv(*prev)
                    prev = cur
                emit_pv(*prev)
                nc.vector.tensor_copy(oT[0:DH, qsl], pv0[0:DH, 0:qw])
                nc.vector.tensor_copy(dsum[0:1, qsl], pv0[DH : DH + 1, 0:qw])
                nc.vector.tensor_copy(oT[DH : 2 * DH, qsl], pv1[64 : 64 + DH, 0:qw])
                nc.vector.tensor_copy(dsum[32:33, qsl], pv1[96:97, 0:qw])
                # 1/denominator for this q-block, overlapped with later blocks
                # (~51-ULP approx; denominators are ~3136 so edge cases moot)
                nc.vector.reciprocal_approx_fast(rdsum[:, qsl], dsum[:, qsl])
                nc.scalar.copy(rdsb[:, qsl], rdsum[:, qsl])

        # ========== phase 3+4: normalize + output projection (Z^T) ==========
        ind33b = persist.tile([33, C], BF16)
        nc.vector.tensor_copy(ind33b[:], ind33[:])
        zT = persist.tile([C, N], F32)
        with tc.tile_pool(name="ps_fin", bufs=3, space="PSUM") as ps_fin, \
             tc.tile_pool(name="otn", bufs=3) as otnp, \
             phase_ctx("p34"):
            def emit_bcast(qo, qw):
                sl = slice(qo, qo + qw)
                bc = ps_fin.tile([C, 512], F32, tag="bc")
                nc.tensor.matmul(
                    bc[:, 0:qw], ind33b[:], rdsb[:, sl], start=True, stop=True
                )
                return bc

            def emit_out(bc, qo, qw):
                sl = slice(qo, qo + qw)
                otn = otnp.tile([INNER, 512], BF16, tag="otn")
                nc.vector.tensor_mul(otn[:, 0:qw], oT[:, sl], bc[:, 0:qw])
                zp = ps_fin.tile([C, 512], F32, tag="zp")
                nc.tensor.matmul(
                    zp[:, 0:qw], wo[:], otn[:, 0:qw], start=True, stop=True
                )
                nc.vector.tensor_scalar_add(zT[:, sl], zp[:, 0:qw], boT[:])

            # software pipeline: bcast(i+1) ahead of the dependent chain of i
            # so the PE never stalls in-order on the DVE normalize multiply.
            prev_fin = None
            for qo, qw in QBLKS:
                bc = emit_bcast(qo, qw)
                if prev_fin is not None:
                    emit_out(*prev_fin)
                prev_fin = (bc, qo, qw)
            emit_out(*prev_fin)
            nc.sync.dma_start(y_d[:], zT[:])

    nc.compile()
    return nc


_CACHED = {}


def _get_kernel():
    if "nc" not in _CACHED:
        _CACHED["nc"] = build_kernel()
    return _CACHED["nc"]


LAST_PERF = {}


def make_in_maps(x, context, Wq, Wk, Wv, Wo, bo):
    import ml_dtypes
    BF = ml_dtypes.bfloat16
    x = np.asarray(x, dtype=np.float32).astype(BF)
    context = np.asarray(context, dtype=np.float32).astype(BF)
    wq = np.ascontiguousarray(
        (np.asarray(Wq, dtype=np.float32) * np.float32(SCALE)).astype(BF))
    wk = np.ascontiguousarray(np.asarray(Wk, dtype=np.float32).astype(BF))
    wv = np.ascontiguousarray(np.asarray(Wv, dtype=np.float32).astype(BF))
    wo = np.ascontiguousarray(np.asarray(Wo, dtype=np.float32).astype(BF))
    boT = np.ascontiguousarray(np.asarray(bo, dtype=np.float32).reshape(C, 1))
    B = x.shape[0]
    in_maps = []
    for b in range(B):
        cTp = np.zeros((C, NKP), BF)
        cTp[:, :NK] = context[b].T
        in_maps.append(
            {
                "xT": np.ascontiguousarray(x[b].T),
                "cT": cTp,
                "wq": wq, "wk": wk, "wv": wv, "wo": wo, "bo": boT,
            }
        )
    return in_maps


def kernel(x, context, Wq, Wk, Wv, Wo, bo, _trace=False):
    in_maps = make_in_maps(x, context, Wq, Wk, Wv, Wo, bo)
    nc = _get_kernel()
    B = len(in_maps)
    res = bass_utils.run_bass_kernel_spmd(
        nc, in_maps, core_ids=list(range(B)), trace=_trace
    )
    LAST_PERF["exec_time_ns"] = res.exec_time_ns
    LAST_PERF["trace"] = res.instructions_and_trace
    # y is Z^T [64, 3136] per core; transpose back while unsharding
    out = np.stack(
        [np.ascontiguousarray(res.results[b]["y"].T) for b in range(B)], axis=0
    )
    return out



# revision 11
# speedup vs baseline: 1.0692x; 1.0692x over previous
"""Trainium2 Bass kernel for LDM-style cross-attention fusion.

Problem (hardcoded shapes):
  x:       [8, 3136, 64]   queries source
  context: [8, 3136, 64]   key/value source
  Wq/Wk/Wv/Wo: [64, 64], bo: [64]
  2 heads x 32 dim, softmax over full 3136x3136 attention matrix.

Sharding: pure data parallel over the batch axis (8 batches -> 8 cores).
Each core computes one batch element entirely on-chip (flash-style: the
[2, 3136, 3136] attention matrix never touches HBM).

Host-side marshalling (pure data movement, no math): x/ctx are sliced per
batch and transposed to channel-major [64, n] (ctx zero-padded to 3200),
so every device DMA moves long contiguous runs (64 descriptors instead of
3072); the kernel returns Z^T [64, 3136] per core and the host transposes
back while unsharding.

Per-core dataflow:
  phase 1: QT = (Wq*scale)^T @ xT, KT = Wk^T @ cT   [64, n] each
           V = ctx @ Wv stored as V_aug [3200, 66] with a ones column per
           head (the PV matmul then also emits the softmax denominator).
  phase 2: for each q-block (448), units u = (k-chunk, head):
              S^T[k,q] = K @ Q^T     (K=32 matmuls, heads in separate PE
                                      row strips via base_partition)
              P^T = exp(S^T)         (ACT, batched over GRP units)
              PV[33, q] += V_aug^T @ P^T   (row 32 = denominator; h1 on
                  psum partitions 64.. -> disjoint PE column strips)
  phase 3: rdsum = 1 / dsum  (dsum pre-memset to 1.0 so untouched rows
           stay finite).
  phase 4: per 128-q-chunk: bcast = ind33^T @ rdsum (outer-product
           broadcast of 1/denom onto the 64 O^T partitions), O^T_n = O^T *
           bcast, Z^T = Wo^T @ O^T_n, + bias via per-partition scalar add.
"""

from contextlib import ExitStack, nullcontext

import numpy as np

import concourse.bass as bass
import concourse.mybir as mybir
import concourse.tile as tile
from concourse import bacc, bass_utils

F32 = mybir.dt.float32
BF16 = mybir.dt.bfloat16
AF = mybir.ActivationFunctionType

N = 3136          # query tokens
NK = 3136         # context tokens
NKP = 3200        # context tokens padded to 25*128
C = 64            # channels
H = 2             # heads
DH = 32           # head dim
INNER = H * DH    # 64
QB = 448          # q block size (free dim of S^T matmuls; fits psum bank)
NQB = N // QB     # 7
KC = 128          # k chunk (partition dim of S^T tiles)
NKC = NKP // KC   # 25
VW = DH + 1       # V columns per head incl. ones column (33)
SCALE = float(DH) ** -0.5
GRP = 3           # exp batch: units per S psum tile (3 banks)

N_CORES = 8


def _ts(i, s):
    return slice(i * s, (i + 1) * s)


def build_kernel(n_cores=N_CORES, repeat=None, grp=GRP, pv_bufs=1, s_bufs=2):
    rep_phase, rep_n = (None, None)
    if isinstance(repeat, tuple):
        rep_phase, rep_n = repeat
    elif repeat:
        rep_phase, rep_n = "all", repeat

    nc = bacc.Bacc(
        "TRN2",
        target_bir_lowering=False,
        debug=False,
        enable_asserts=False,
        num_devices=n_cores,
    )
    xT_d = nc.dram_tensor("xT", [C, N], BF16, kind="ExternalInput").ap()
    cT_d = nc.dram_tensor("cT", [C, NKP], BF16, kind="ExternalInput").ap()
    wq_d = nc.dram_tensor("wq", [C, INNER], BF16, kind="ExternalInput").ap()
    wk_d = nc.dram_tensor("wk", [C, INNER], BF16, kind="ExternalInput").ap()
    wv_d = nc.dram_tensor("wv", [C, INNER], BF16, kind="ExternalInput").ap()
    wo_d = nc.dram_tensor("wo", [INNER, C], BF16, kind="ExternalInput").ap()
    bo_d = nc.dram_tensor("bo", [C, 1], F32, kind="ExternalInput").ap()
    y_d = nc.dram_tensor("y", [C, N], F32, kind="ExternalOutput").ap()

    with tile.TileContext(nc) as tc, ExitStack() as ctx:

        def phase_ctx(name):
            return tc.For_i(0, rep_n, 1) if rep_phase == name else nullcontext()

        if rep_phase == "all":
            ctx.enter_context(tc.For_i(0, rep_n, 1))
        persist = ctx.enter_context(tc.tile_pool(name="persist", bufs=1))

        # ---- constants + inputs ----
        wq = persist.tile([C, INNER], BF16)
        nc.sync.dma_start(wq[:], wq_d[:])
        wk = persist.tile([C, INNER], BF16)
        nc.sync.dma_start(wk[:], wk_d[:])
        wv = persist.tile([C, INNER], BF16)
        nc.sync.dma_start(wv[:], wv_d[:])
        wo = persist.tile([INNER, C], BF16)
        nc.sync.dma_start(wo[:], wo_d[:])
        boT = persist.tile([C, 1], F32)
        nc.sync.dma_start(boT[:], bo_d[:])
        xT = persist.tile([C, N], BF16)
        nc.sync.dma_start(xT[:], xT_d[:])
        cT = persist.tile([C, NKP], BF16)
        nc.sync.dma_start(cT[:], cT_d[:])

        # indicator [33, 64]: row 0 -> head-0 cols, row 32 -> head-1 cols
        ind33 = persist.tile([33, C], F32)
        nc.vector.memset(ind33[:], 0.0)
        nc.vector.memset(ind33[0:1, 0:DH], 1.0)
        nc.vector.memset(ind33[32:33, DH : 2 * DH], 1.0)

        # ================= phase 1: projections =================
        qT = persist.tile([INNER, N], BF16)    # Q^T (pre-scaled via host wq)
        kT = persist.tile([INNER, NKP], BF16)  # K^T
        vaug = persist.tile([128, NKC * 2 * VW], BF16)  # [h0 32|1|h1 32|1] per chunk
        with tc.tile_pool(name="ps_proj", bufs=3, space="PSUM") as ps_proj, \
             tc.tile_pool(name="ps_v", bufs=2, space="PSUM") as ps_v, \
             phase_ctx("p1"):
            # interleave Q^T (copies on DVE) and K^T (copies on ACT) blocks so
            # both copy engines drain the proj psums concurrently
            for qb in range(NQB + 1):
                if qb < NQB:
                    pp = ps_proj.tile([INNER, QB], F32, tag="pq")
                    nc.tensor.matmul(
                        pp[:], wq[:], xT[:, _ts(qb, QB)], start=True, stop=True
                    )
                    nc.vector.tensor_copy(qT[:, _ts(qb, QB)], pp[:])
                # K^T over padded width (cT pad columns are zero from host)
                w = min(QB, NKP - qb * QB)
                if w > 0:
                    pp = ps_proj.tile([INNER, QB], F32, tag="pk")
                    nc.tensor.matmul(
                        pp[:, 0:w], wk[:], cT[:, qb * QB : qb * QB + w],
                        start=True, stop=True,
                    )
                    nc.scalar.copy(kT[:, qb * QB : qb * QB + w], pp[:, 0:w])
            # V chunks -> vaug (data cols; ones cols via memset)
            for k in range(NKC):
                pv = ps_v.tile([128, INNER], F32, tag="pv")
                nc.tensor.matmul(pv[:], cT[:, _ts(k, 128)], wv[:], start=True, stop=True)
                base = k * 2 * VW
                dst_ap = vaug[:, base : base + 2 * VW].rearrange(
                    "p (h w) -> p h w", h=2
                )[:, :, 0:DH]
                src_ap = pv[:].rearrange("p (h w) -> p h w", h=2)
                nc.vector.tensor_copy(dst_ap, src_ap)
            ones_ap = vaug[:].rearrange("p (k h w) -> p k h w", k=NKC, h=2)[
                :, 0 : NKC - 1, :, DH : DH + 1
            ]
            nc.vector.memset(ones_ap, 1.0)
            last = vaug[:, (NKC - 1) * 2 * VW :].rearrange("p (h w) -> p h w", h=2)
            nc.vector.memset(last[0:64, :, DH : DH + 1], 1.0)
            nc.vector.memset(last[64:128, :, DH : DH + 1], 0.0)

        # ================= phase 2: attention =================
        # q blocks of 512 (exactly one psum bank) with a ragged 64 tail
        QBLKS = [(i * 512, 512) for i in range(N // 512)]
        if N % 512:
            QBLKS.append(((N // 512) * 512, N % 512))
        oT = persist.tile([INNER, N], F32)    # unnormalized O^T (both heads)
        dsum = persist.tile([33, N], F32)     # rows 0 / 32: denom per head
        rdsum = persist.tile([33, N], F32)    # 1/denom, filled per q-block
        rdsb = persist.tile([33, N], BF16)    # bf16 copy for the bcast matmul
        nc.vector.memset(dsum[:], 1.0)        # keep untouched rows finite
        NU = 2 * NKC  # 50 units per q block
        with tc.tile_pool(name="ps_s", bufs=s_bufs, space="PSUM") as ps_s, \
             tc.tile_pool(name="ps_pv0", bufs=pv_bufs, space="PSUM") as ps_pv0, \
             tc.tile_pool(name="ps_pv1", bufs=pv_bufs, space="PSUM") as ps_pv1, \
             tc.tile_pool(name="pt", bufs=4) as ptp, \
             phase_ctx("p2"):
            for qo, qw in QBLKS:
                qsl = slice(qo, qo + qw)
                pv0 = ps_pv0.tile([VW, 512], F32, tag="pvacc0")
                pv1 = ps_pv1.tile([64 + VW, 512], F32, tag="pvacc1")
                pvout = [pv0[0:VW, 0:qw], pv1[64 : 64 + VW, 0:qw]]

                def emit_s_exp(g):
                    units = [g * grp + j for j in range(grp) if g * grp + j < NU]
                    nu = len(units)
                    s = ps_s.tile([128, grp * 512], F32, tag="s")
                    for j, u in enumerate(units):
                        k, h = divmod(u, 2)
                        nc.tensor.matmul(
                            s[:, j * 512 : j * 512 + qw],
                            kT[_ts(h, DH), _ts(k, 128)],
                            qT[_ts(h, DH), qsl],
                            start=True, stop=True,
                        )
                    p = ptp.tile([128, grp * 512], BF16, tag="p")
                    nc.scalar.activation(
                        p[:].rearrange("q (j w) -> q j w", w=512)[:, 0:nu, 0:qw],
                        s[:].rearrange("q (j w) -> q j w", w=512)[:, 0:nu, 0:qw],
                        AF.Exp,
                    )
                    return units, p

                def emit_pv(units, p):
                    for j, u in enumerate(units):
                        k, h = divmod(u, 2)
                        base = k * 2 * VW + h * VW
                        nc.tensor.matmul(
                            pvout[h],
                            vaug[:, base : base + VW],
                            p[:, j * 512 : j * 512 + qw],
                            start=(k == 0), stop=(k == NKC - 1),
                        )

                # software pipeline: keep S(g+1) ahead of PV(g) in the PE
                # stream so the PE streams S while ACT runs exp(g), instead
                # of stalling in-order on the exp -> PV dependency.
                ngr = (NU + grp - 1) // grp
                pend = [emit_s_exp(0), emit_s_exp(1)]
                for g in range(2, ngr):
                    pend.append(emit_s_exp(g))
                    emit_pv(*pend.pop(0))
                while pend:
                    emit_pv(*pend.pop(0))
                nc.vector.tensor_copy(oT[0:DH, qsl], pv0[0:DH, 0:qw])
                nc.vector.tensor_copy(dsum[0:1, qsl], pv0[DH : DH + 1, 0:qw])
                nc.vector.tensor_copy(oT[DH : 2 * DH, qsl], pv1[64 : 64 + DH, 0:qw])
                nc.vector.tensor_copy(dsum[32:33, qsl], pv1[96:97, 0:qw])
                # 1/denominator for this q-block, overlapped with later blocks
                # (~51-ULP approx; denominators are ~3136 so edge cases moot)
                nc.vector.reciprocal_approx_fast(rdsum[:, qsl], dsum[:, qsl])
                nc.scalar.copy(rdsb[:, qsl], rdsum[:, qsl])

        # ========== phase 3+4: normalize + output projection (Z^T) ==========
        ind33b = persist.tile([33, C], BF16)
        nc.vector.tensor_copy(ind33b[:], ind33[:])
        zT = persist.tile([C, N], F32)
        with tc.tile_pool(name="ps_fin", bufs=3, space="PSUM") as ps_fin, \
             tc.tile_pool(name="otn", bufs=3) as otnp, \
             phase_ctx("p34"):
            def emit_bcast(qo, qw):
                sl = slice(qo, qo + qw)
                bc = ps_fin.tile([C, 512], F32, tag="bc")
                nc.tensor.matmul(
                    bc[:, 0:qw], ind33b[:], rdsb[:, sl], start=True, stop=True
                )
                return bc

            def emit_out(bc, qo, qw):
                sl = slice(qo, qo + qw)
                otn = otnp.tile([INNER, 512], BF16, tag="otn")
                nc.vector.tensor_mul(otn[:, 0:qw], oT[:, sl], bc[:, 0:qw])
                zp = ps_fin.tile([C, 512], F32, tag="zp")
                nc.tensor.matmul(
                    zp[:, 0:qw], wo[:], otn[:, 0:qw], start=True, stop=True
                )
                nc.scalar.activation(
                    zT[:, sl], zp[:, 0:qw], AF.Identity, bias=boT[:]
                )

            # software pipeline: bcast(i+1) ahead of the dependent chain of i
            # so the PE never stalls in-order on the DVE normalize multiply.
            prev_fin = None
            for qo, qw in QBLKS:
                bc = emit_bcast(qo, qw)
                if prev_fin is not None:
                    emit_out(*prev_fin)
                prev_fin = (bc, qo, qw)
            emit_out(*prev_fin)
            nc.sync.dma_start(y_d[:], zT[:])

    nc.compile()
    return nc


_CACHED = {}


def _get_kernel():
    if "nc" not in _CACHED:
        _CACHED["nc"] = build_kernel()
    return _CACHED["nc"]


LAST_PERF = {}


def make_in_maps(x, context, Wq, Wk, Wv, Wo, bo):
    import ml_dtypes
    BF = ml_dtypes.bfloat16
    x = np.asarray(x, dtype=np.float32).astype(BF)
    context = np.asarray(context, dtype=np.float32).astype(BF)
    wq = np.ascontiguousarray(
        (np.asarray(Wq, dtype=np.float32) * np.float32(SCALE)).astype(BF))
    wk = np.ascontiguousarray(np.asarray(Wk, dtype=np.float32).astype(BF))
    wv = np.ascontiguousarray(np.asarray(Wv, dtype=np.float32).astype(BF))
    wo = np.ascontiguousarray(np.asarray(Wo, dtype=np.float32).astype(BF))
    boT = np.ascontiguousarray(np.asarray(bo, dtype=np.float32).reshape(C, 1))
    B = x.shape[0]
    in_maps = []
    for b in range(B):
        cTp = np.zeros((C, NKP), BF)
        cTp[:, :NK] = context[b].T
        in_maps.append(
            {
                "xT": np.ascontiguousarray(x[b].T),
                "cT": cTp,
                "wq": wq, "wk": wk, "wv": wv, "wo": wo, "bo": boT,
            }
        )
    return in_maps


def kernel(x, context, Wq, Wk, Wv, Wo, bo, _trace=False):
    in_maps = make_in_maps(x, context, Wq, Wk, Wv, Wo, bo)
    nc = _get_kernel()
    B = len(in_maps)
    res = bass_utils.run_bass_kernel_spmd(
        nc, in_maps, core_ids=list(range(B)), trace=_trace
    )
    LAST_PERF["exec_time_ns"] = res.exec_time_ns
    LAST_PERF["trace"] = res.instructions_and_trace
    # y is Z^T [64, 3136] per core; transpose back while unsharding
    out = np.stack(
        [np.ascontiguousarray(res.results[b]["y"].T) for b in range(B)], axis=0
    )
    return out



# revision 12
# speedup vs baseline: 1.1823x; 1.1058x over previous
"""Trainium2 Bass kernel for LDM-style cross-attention fusion.

Problem (hardcoded shapes):
  x:       [8, 3136, 64]   queries source
  context: [8, 3136, 64]   key/value source
  Wq/Wk/Wv/Wo: [64, 64], bo: [64]
  2 heads x 32 dim, softmax over full 3136x3136 attention matrix.

Sharding: pure data parallel over the batch axis (8 batches -> 8 cores).
Each core computes one batch element entirely on-chip (flash-style: the
[2, 3136, 3136] attention matrix never touches HBM).

Host-side marshalling (pure data movement, no math): x/ctx are sliced per
batch and transposed to channel-major [64, n] (ctx zero-padded to 3200),
so every device DMA moves long contiguous runs (64 descriptors instead of
3072); the kernel returns Z^T [64, 3136] per core and the host transposes
back while unsharding.

Per-core dataflow:
  phase 1: QT = (Wq*scale)^T @ xT, KT = Wk^T @ cT   [64, n] each
           V = ctx @ Wv stored as V_aug [3200, 66] with a ones column per
           head (the PV matmul then also emits the softmax denominator).
  phase 2: for each q-block (448), units u = (k-chunk, head):
              S^T[k,q] = K @ Q^T     (K=32 matmuls, heads in separate PE
                                      row strips via base_partition)
              P^T = exp(S^T)         (ACT, batched over GRP units)
              PV[33, q] += V_aug^T @ P^T   (row 32 = denominator; h1 on
                  psum partitions 64.. -> disjoint PE column strips)
  phase 3: rdsum = 1 / dsum  (dsum pre-memset to 1.0 so untouched rows
           stay finite).
  phase 4: per 128-q-chunk: bcast = ind33^T @ rdsum (outer-product
           broadcast of 1/denom onto the 64 O^T partitions), O^T_n = O^T *
           bcast, Z^T = Wo^T @ O^T_n, + bias via per-partition scalar add.
"""

from contextlib import ExitStack, nullcontext

import numpy as np

import concourse.bass as bass
import concourse.mybir as mybir
import concourse.tile as tile
from concourse import bacc, bass_utils

F32 = mybir.dt.float32
BF16 = mybir.dt.bfloat16
AF = mybir.ActivationFunctionType

N = 3136          # query tokens
NK = 3136         # context tokens
NKP = 3200        # context tokens padded to 25*128
C = 64            # channels
H = 2             # heads
DH = 32           # head dim
INNER = H * DH    # 64
QB = 448          # q block size (free dim of S^T matmuls; fits psum bank)
NQB = N // QB     # 7
KC = 128          # k chunk (partition dim of S^T tiles)
NKC = NKP // KC   # 25
VW = DH + 1       # V columns per head incl. ones column (33)
SCALE = float(DH) ** -0.5
GRP = 2           # exp batch: units per S psum tile (2 banks x 3 bufs)

N_CORES = 8


def _ts(i, s):
    return slice(i * s, (i + 1) * s)


def build_kernel(n_cores=N_CORES, repeat=None, grp=GRP, pv_bufs=1, s_bufs=3):
    rep_phase, rep_n = (None, None)
    if isinstance(repeat, tuple):
        rep_phase, rep_n = repeat
    elif repeat:
        rep_phase, rep_n = "all", repeat

    nc = bacc.Bacc(
        "TRN2",
        target_bir_lowering=False,
        debug=False,
        enable_asserts=False,
        num_devices=n_cores,
    )
    xT_d = nc.dram_tensor("xT", [C, N], BF16, kind="ExternalInput").ap()
    cT_d = nc.dram_tensor("cT", [C, NKP], BF16, kind="ExternalInput").ap()
    wq_d = nc.dram_tensor("wq", [C, INNER], BF16, kind="ExternalInput").ap()
    wk_d = nc.dram_tensor("wk", [C, INNER], BF16, kind="ExternalInput").ap()
    wv_d = nc.dram_tensor("wv", [C, INNER], BF16, kind="ExternalInput").ap()
    wo_d = nc.dram_tensor("wo", [INNER, C], BF16, kind="ExternalInput").ap()
    bo_d = nc.dram_tensor("bo", [C, 1], F32, kind="ExternalInput").ap()
    y_d = nc.dram_tensor("y", [C, N], F32, kind="ExternalOutput").ap()

    with tile.TileContext(nc) as tc, ExitStack() as ctx:

        def phase_ctx(name):
            return tc.For_i(0, rep_n, 1) if rep_phase == name else nullcontext()

        if rep_phase == "all":
            ctx.enter_context(tc.For_i(0, rep_n, 1))
        persist = ctx.enter_context(tc.tile_pool(name="persist", bufs=1))

        # ---- constants + inputs ----
        wq = persist.tile([C, INNER], BF16)
        nc.sync.dma_start(wq[:], wq_d[:])
        wk = persist.tile([C, INNER], BF16)
        nc.sync.dma_start(wk[:], wk_d[:])
        wv = persist.tile([C, INNER], BF16)
        nc.sync.dma_start(wv[:], wv_d[:])
        wo = persist.tile([INNER, C], BF16)
        nc.sync.dma_start(wo[:], wo_d[:])
        boT = persist.tile([C, 1], F32)
        nc.sync.dma_start(boT[:], bo_d[:])
        xT = persist.tile([C, N], BF16)
        nc.sync.dma_start(xT[:], xT_d[:])
        cT = persist.tile([C, NKP], BF16)
        nc.sync.dma_start(cT[:], cT_d[:])

        # indicator [33, 64]: row 0 -> head-0 cols, row 32 -> head-1 cols
        ind33 = persist.tile([33, C], F32)
        nc.vector.memset(ind33[:], 0.0)
        nc.vector.memset(ind33[0:1, 0:DH], 1.0)
        nc.vector.memset(ind33[32:33, DH : 2 * DH], 1.0)

        # ================= phase 1: projections =================
        qT = persist.tile([INNER, N], BF16)    # Q^T (pre-scaled via host wq)
        kT = persist.tile([INNER, NKP], BF16)  # K^T
        vaug = persist.tile([128, NKC * 2 * VW], BF16)  # [h0 32|1|h1 32|1] per chunk
        with tc.tile_pool(name="ps_proj", bufs=3, space="PSUM") as ps_proj, \
             tc.tile_pool(name="ps_v", bufs=2, space="PSUM") as ps_v, \
             phase_ctx("p1"):
            # interleave Q^T (copies on DVE) and K^T (copies on ACT) blocks so
            # both copy engines drain the proj psums concurrently
            for qb in range(NQB + 1):
                if qb < NQB:
                    pp = ps_proj.tile([INNER, QB], F32, tag="pq")
                    nc.tensor.matmul(
                        pp[:], wq[:], xT[:, _ts(qb, QB)], start=True, stop=True
                    )
                    nc.vector.tensor_copy(qT[:, _ts(qb, QB)], pp[:])
                # K^T over padded width (cT pad columns are zero from host)
                w = min(QB, NKP - qb * QB)
                if w > 0:
                    pp = ps_proj.tile([INNER, QB], F32, tag="pk")
                    nc.tensor.matmul(
                        pp[:, 0:w], wk[:], cT[:, qb * QB : qb * QB + w],
                        start=True, stop=True,
                    )
                    nc.scalar.copy(kT[:, qb * QB : qb * QB + w], pp[:, 0:w])
            # V chunks -> vaug (data cols; ones cols via memset)
            for k in range(NKC):
                pv = ps_v.tile([128, INNER], F32, tag="pv")
                nc.tensor.matmul(pv[:], cT[:, _ts(k, 128)], wv[:], start=True, stop=True)
                base = k * 2 * VW
                dst_ap = vaug[:, base : base + 2 * VW].rearrange(
                    "p (h w) -> p h w", h=2
                )[:, :, 0:DH]
                src_ap = pv[:].rearrange("p (h w) -> p h w", h=2)
                nc.vector.tensor_copy(dst_ap, src_ap)
            ones_ap = vaug[:].rearrange("p (k h w) -> p k h w", k=NKC, h=2)[
                :, 0 : NKC - 1, :, DH : DH + 1
            ]
            nc.vector.memset(ones_ap, 1.0)
            last = vaug[:, (NKC - 1) * 2 * VW :].rearrange("p (h w) -> p h w", h=2)
            nc.vector.memset(last[0:64, :, DH : DH + 1], 1.0)
            nc.vector.memset(last[64:128, :, DH : DH + 1], 0.0)

        # ================= phase 2: attention =================
        # q blocks of 512 (exactly one psum bank) with a ragged 64 tail
        QBLKS = [(i * 512, 512) for i in range(N // 512)]
        if N % 512:
            QBLKS.append(((N // 512) * 512, N % 512))
        oT = persist.tile([INNER, N], F32)    # unnormalized O^T (both heads)
        dsum = persist.tile([33, N], F32)     # rows 0 / 32: denom per head
        rdsum = persist.tile([33, N], F32)    # 1/denom, filled per q-block
        rdsb = persist.tile([33, N], BF16)    # bf16 copy for the bcast matmul
        nc.vector.memset(dsum[:], 1.0)        # keep untouched rows finite
        NU = 2 * NKC  # 50 units per q block
        with tc.tile_pool(name="ps_s", bufs=s_bufs, space="PSUM") as ps_s, \
             tc.tile_pool(name="ps_pv0", bufs=pv_bufs, space="PSUM") as ps_pv0, \
             tc.tile_pool(name="ps_pv1", bufs=pv_bufs, space="PSUM") as ps_pv1, \
             tc.tile_pool(name="pt", bufs=4) as ptp, \
             phase_ctx("p2"):
            for qo, qw in QBLKS:
                qsl = slice(qo, qo + qw)
                pv0 = ps_pv0.tile([VW, 512], F32, tag="pvacc0")
                pv1 = ps_pv1.tile([64 + VW, 512], F32, tag="pvacc1")
                pvout = [pv0[0:VW, 0:qw], pv1[64 : 64 + VW, 0:qw]]

                def emit_s_exp(g):
                    units = [g * grp + j for j in range(grp) if g * grp + j < NU]
                    nu = len(units)
                    s = ps_s.tile([128, grp * 512], F32, tag="s")
                    for j, u in enumerate(units):
                        k, h = divmod(u, 2)
                        nc.tensor.matmul(
                            s[:, j * 512 : j * 512 + qw],
                            kT[_ts(h, DH), _ts(k, 128)],
                            qT[_ts(h, DH), qsl],
                            start=True, stop=True,
                        )
                    p = ptp.tile([128, grp * 512], BF16, tag="p")
                    nc.scalar.activation(
                        p[:].rearrange("q (j w) -> q j w", w=512)[:, 0:nu, 0:qw],
                        s[:].rearrange("q (j w) -> q j w", w=512)[:, 0:nu, 0:qw],
                        AF.Exp,
                    )
                    return units, p

                def emit_pv(units, p):
                    for j, u in enumerate(units):
                        k, h = divmod(u, 2)
                        base = k * 2 * VW + h * VW
                        nc.tensor.matmul(
                            pvout[h],
                            vaug[:, base : base + VW],
                            p[:, j * 512 : j * 512 + qw],
                            start=(k == 0), stop=(k == NKC - 1),
                        )

                # software pipeline: keep S(g+1) ahead of PV(g) in the PE
                # stream so the PE streams S while ACT runs exp(g), instead
                # of stalling in-order on the exp -> PV dependency.
                ngr = (NU + grp - 1) // grp
                pend = [emit_s_exp(0), emit_s_exp(1)]
                for g in range(2, ngr):
                    pend.append(emit_s_exp(g))
                    emit_pv(*pend.pop(0))
                while pend:
                    emit_pv(*pend.pop(0))
                nc.vector.tensor_copy(oT[0:DH, qsl], pv0[0:DH, 0:qw])
                nc.vector.tensor_copy(dsum[0:1, qsl], pv0[DH : DH + 1, 0:qw])
                nc.vector.tensor_copy(oT[DH : 2 * DH, qsl], pv1[64 : 64 + DH, 0:qw])
                nc.vector.tensor_copy(dsum[32:33, qsl], pv1[96:97, 0:qw])
                # 1/denominator for this q-block, overlapped with later blocks
                # (~51-ULP approx; denominators are ~3136 so edge cases moot)
                nc.vector.reciprocal_approx_fast(rdsum[:, qsl], dsum[:, qsl])
                nc.scalar.copy(rdsb[:, qsl], rdsum[:, qsl])

        # ========== phase 3+4: normalize + output projection (Z^T) ==========
        ind33b = persist.tile([33, C], BF16)
        nc.vector.tensor_copy(ind33b[:], ind33[:])
        zT = persist.tile([C, N], F32)
        with tc.tile_pool(name="ps_fin", bufs=3, space="PSUM") as ps_fin, \
             tc.tile_pool(name="otn", bufs=3) as otnp, \
             phase_ctx("p34"):
            def emit_bcast(qo, qw):
                sl = slice(qo, qo + qw)
                bc = ps_fin.tile([C, 512], F32, tag="bc")
                nc.tensor.matmul(
                    bc[:, 0:qw], ind33b[:], rdsb[:, sl], start=True, stop=True
                )
                return bc

            def emit_out(bc, qo, qw):
                sl = slice(qo, qo + qw)
                otn = otnp.tile([INNER, 512], BF16, tag="otn")
                nc.vector.tensor_mul(otn[:, 0:qw], oT[:, sl], bc[:, 0:qw])
                zp = ps_fin.tile([C, 512], F32, tag="zp")
                nc.tensor.matmul(
                    zp[:, 0:qw], wo[:], otn[:, 0:qw], start=True, stop=True
                )
                nc.scalar.activation(
                    zT[:, sl], zp[:, 0:qw], AF.Identity, bias=boT[:]
                )

            # software pipeline: bcast(i+1) ahead of the dependent chain of i
            # so the PE never stalls in-order on the DVE normalize multiply.
            prev_fin = None
            for qo, qw in QBLKS:
                bc = emit_bcast(qo, qw)
                if prev_fin is not None:
                    emit_out(*prev_fin)
                prev_fin = (bc, qo, qw)
            emit_out(*prev_fin)
            nc.sync.dma_start(y_d[:], zT[:])

    nc.compile()
    return nc


_CACHED = {}


def _get_kernel():
    if "nc" not in _CACHED:
        _CACHED["nc"] = build_kernel()
    return _CACHED["nc"]


LAST_PERF = {}


def make_in_maps(x, context, Wq, Wk, Wv, Wo, bo):
    import ml_dtypes
    BF = ml_dtypes.bfloat16
    x = np.asarray(x, dtype=np.float32).astype(BF)
    context = np.asarray(context, dtype=np.float32).astype(BF)
    wq = np.ascontiguousarray(
        (np.asarray(Wq, dtype=np.float32) * np.float32(SCALE)).astype(BF))
    wk = np.ascontiguousarray(np.asarray(Wk, dtype=np.float32).astype(BF))
    wv = np.ascontiguousarray(np.asarray(Wv, dtype=np.float32).astype(BF))
    wo = np.ascontiguousarray(np.asarray(Wo, dtype=np.float32).astype(BF))
    boT = np.ascontiguousarray(np.asarray(bo, dtype=np.float32).reshape(C, 1))
    B = x.shape[0]
    in_maps = []
    for b in range(B):
        cTp = np.zeros((C, NKP), BF)
        cTp[:, :NK] = context[b].T
        in_maps.append(
            {
                "xT": np.ascontiguousarray(x[b].T),
                "cT": cTp,
                "wq": wq, "wk": wk, "wv": wv, "wo": wo, "bo": boT,
            }
        )
    return in_maps


def kernel(x, context, Wq, Wk, Wv, Wo, bo, _trace=False):
    in_maps = make_in_maps(x, context, Wq, Wk, Wv, Wo, bo)
    nc = _get_kernel()
    B = len(in_maps)
    res = bass_utils.run_bass_kernel_spmd(
        nc, in_maps, core_ids=list(range(B)), trace=_trace
    )
    LAST_PERF["exec_time_ns"] = res.exec_time_ns
    LAST_PERF["trace"] = res.instructions_and_trace
    # y is Z^T [64, 3136] per core; transpose back while unsharding
    out = np.stack(
        [np.ascontiguousarray(res.results[b]["y"].T) for b in range(B)], axis=0
    )
    return out



# revision 13
# speedup vs baseline: 1.2228x; 1.0343x over previous
"""Trainium2 Bass kernel for LDM-style cross-attention fusion.

Problem (hardcoded shapes):
  x:       [8, 3136, 64]   queries source
  context: [8, 3136, 64]   key/value source
  Wq/Wk/Wv/Wo: [64, 64], bo: [64]
  2 heads x 32 dim, softmax over full 3136x3136 attention matrix.

Sharding: pure data parallel over the batch axis (8 batches -> 8 cores).
Each core computes one batch element entirely on-chip (flash-style: the
[2, 3136, 3136] attention matrix never touches HBM).

Host-side marshalling (pure data movement, no math): x/ctx are sliced per
batch and transposed to channel-major [64, n] (ctx zero-padded to 3200),
so every device DMA moves long contiguous runs (64 descriptors instead of
3072); the kernel returns Z^T [64, 3136] per core and the host transposes
back while unsharding.

Per-core dataflow:
  phase 1: QT = (Wq*scale)^T @ xT, KT = Wk^T @ cT   [64, n] each
           V = ctx @ Wv stored as V_aug [3200, 66] with a ones column per
           head (the PV matmul then also emits the softmax denominator).
  phase 2: for each q-block (448), units u = (k-chunk, head):
              S^T[k,q] = K @ Q^T     (K=32 matmuls, heads in separate PE
                                      row strips via base_partition)
              P^T = exp(S^T)         (ACT, batched over GRP units)
              PV[33, q] += V_aug^T @ P^T   (row 32 = denominator; h1 on
                  psum partitions 64.. -> disjoint PE column strips)
  phase 3: rdsum = 1 / dsum  (dsum pre-memset to 1.0 so untouched rows
           stay finite).
  phase 4: per 128-q-chunk: bcast = ind33^T @ rdsum (outer-product
           broadcast of 1/denom onto the 64 O^T partitions), O^T_n = O^T *
           bcast, Z^T = Wo^T @ O^T_n, + bias via per-partition scalar add.
"""

from contextlib import ExitStack, nullcontext

import numpy as np

import concourse.bass as bass
import concourse.mybir as mybir
import concourse.tile as tile
from concourse import bacc, bass_utils

F32 = mybir.dt.float32
BF16 = mybir.dt.bfloat16
AF = mybir.ActivationFunctionType

N = 3136          # query tokens
NK = 3136         # context tokens
NKP = 3200        # context tokens padded to 25*128
C = 64            # channels
H = 2             # heads
DH = 32           # head dim
INNER = H * DH    # 64
QB = 448          # q block size (free dim of S^T matmuls; fits psum bank)
NQB = N // QB     # 7
KC = 128          # k chunk (partition dim of S^T tiles)
NKC = NKP // KC   # 25
VW = DH + 1       # V columns per head incl. ones column (33)
SCALE = float(DH) ** -0.5
GRP = 2           # exp batch: units per S psum tile (2 banks x 3 bufs)

N_CORES = 8


def _ts(i, s):
    return slice(i * s, (i + 1) * s)


def build_kernel(n_cores=N_CORES, repeat=None, grp=GRP, pv_bufs=1, s_bufs=3):
    rep_phase, rep_n = (None, None)
    if isinstance(repeat, tuple):
        rep_phase, rep_n = repeat
    elif repeat:
        rep_phase, rep_n = "all", repeat

    nc = bacc.Bacc(
        "TRN2",
        target_bir_lowering=False,
        debug=False,
        enable_asserts=False,
        num_devices=n_cores,
    )
    xT_d = nc.dram_tensor("xT", [C, N], BF16, kind="ExternalInput").ap()
    cT_d = nc.dram_tensor("cT", [C, NKP], BF16, kind="ExternalInput").ap()
    wq_d = nc.dram_tensor("wq", [C, INNER], BF16, kind="ExternalInput").ap()
    wk_d = nc.dram_tensor("wk", [C, INNER], BF16, kind="ExternalInput").ap()
    wv_d = nc.dram_tensor("wv", [C, INNER], BF16, kind="ExternalInput").ap()
    wo_d = nc.dram_tensor("wo", [INNER, C], BF16, kind="ExternalInput").ap()
    bo_d = nc.dram_tensor("bo", [C, 1], F32, kind="ExternalInput").ap()
    y_d = nc.dram_tensor("y", [C, N], F32, kind="ExternalOutput").ap()

    with tile.TileContext(nc) as tc, ExitStack() as ctx:

        def phase_ctx(name):
            return tc.For_i(0, rep_n, 1) if rep_phase == name else nullcontext()

        if rep_phase == "all":
            ctx.enter_context(tc.For_i(0, rep_n, 1))
        persist = ctx.enter_context(tc.tile_pool(name="persist", bufs=1))

        # ---- constants + inputs ----
        wq = persist.tile([C, INNER], BF16)
        nc.sync.dma_start(wq[:], wq_d[:])
        wk = persist.tile([C, INNER], BF16)
        nc.sync.dma_start(wk[:], wk_d[:])
        wv = persist.tile([C, INNER], BF16)
        nc.sync.dma_start(wv[:], wv_d[:])
        wo = persist.tile([INNER, C], BF16)
        nc.sync.dma_start(wo[:], wo_d[:])
        boT = persist.tile([C, 1], F32)
        nc.sync.dma_start(boT[:], bo_d[:])
        xT = persist.tile([C, N], BF16)
        nc.sync.dma_start(xT[:], xT_d[:])
        cT = persist.tile([C, NKP], BF16)
        nc.sync.dma_start(cT[:], cT_d[:])

        # indicator [33, 64]: row 0 -> head-0 cols, row 32 -> head-1 cols
        ind33 = persist.tile([33, C], F32)
        nc.vector.memset(ind33[:], 0.0)
        nc.vector.memset(ind33[0:1, 0:DH], 1.0)
        nc.vector.memset(ind33[32:33, DH : 2 * DH], 1.0)

        # ================= phase 1: projections =================
        qT = persist.tile([INNER, N], BF16)    # Q^T (pre-scaled via host wq)
        kT = persist.tile([INNER, NKP], BF16)  # K^T
        vaug = persist.tile([128, NKC * 2 * VW], BF16)  # [h0 32|1|h1 32|1] per chunk
        with tc.tile_pool(name="ps_proj", bufs=3, space="PSUM") as ps_proj, \
             tc.tile_pool(name="ps_v", bufs=2, space="PSUM") as ps_v, \
             phase_ctx("p1"):
            # interleave Q^T (copies on DVE) and K^T (copies on ACT) blocks so
            # both copy engines drain the proj psums concurrently
            for qb in range(NQB + 1):
                if qb < NQB:
                    pp = ps_proj.tile([INNER, QB], F32, tag="pq")
                    nc.tensor.matmul(
                        pp[:], wq[:], xT[:, _ts(qb, QB)], start=True, stop=True
                    )
                    nc.vector.tensor_copy(qT[:, _ts(qb, QB)], pp[:])
                # K^T over padded width (cT pad columns are zero from host)
                w = min(QB, NKP - qb * QB)
                if w > 0:
                    pp = ps_proj.tile([INNER, QB], F32, tag="pk")
                    nc.tensor.matmul(
                        pp[:, 0:w], wk[:], cT[:, qb * QB : qb * QB + w],
                        start=True, stop=True,
                    )
                    nc.scalar.copy(kT[:, qb * QB : qb * QB + w], pp[:, 0:w])
            # V chunks -> vaug (data cols; ones cols via memset)
            for k in range(NKC):
                pv = ps_v.tile([128, INNER], F32, tag="pv")
                nc.tensor.matmul(pv[:], cT[:, _ts(k, 128)], wv[:], start=True, stop=True)
                base = k * 2 * VW
                dst_ap = vaug[:, base : base + 2 * VW].rearrange(
                    "p (h w) -> p h w", h=2
                )[:, :, 0:DH]
                src_ap = pv[:].rearrange("p (h w) -> p h w", h=2)
                nc.vector.tensor_copy(dst_ap, src_ap)
            ones_ap = vaug[:].rearrange("p (k h w) -> p k h w", k=NKC, h=2)[
                :, 0 : NKC - 1, :, DH : DH + 1
            ]
            nc.vector.memset(ones_ap, 1.0)
            last = vaug[:, (NKC - 1) * 2 * VW :].rearrange("p (h w) -> p h w", h=2)
            nc.vector.memset(last[0:64, :, DH : DH + 1], 1.0)
            nc.vector.memset(last[64:128, :, DH : DH + 1], 0.0)

        # ================= phase 2: attention =================
        # q blocks of 512 (exactly one psum bank) with a ragged 64 tail
        QBLKS = [(i * 512, 512) for i in range(N // 512)]
        if N % 512:
            QBLKS.append(((N // 512) * 512, N % 512))
        oT = persist.tile([INNER, N], F32)    # unnormalized O^T (both heads)
        dsum = persist.tile([33, N], F32)     # rows 0 / 32: denom per head
        rdsum = persist.tile([33, N], F32)    # 1/denom, filled per q-block
        rdsb = persist.tile([33, N], BF16)    # bf16 copy for the bcast matmul
        nc.vector.memset(dsum[:], 1.0)        # keep untouched rows finite
        NU = 2 * NKC  # 50 units per q block
        with tc.tile_pool(name="ps_s", bufs=s_bufs, space="PSUM") as ps_s, \
             tc.tile_pool(name="ps_pv", bufs=2, space="PSUM") as ps_pv, \
             tc.tile_pool(name="pt", bufs=4) as ptp, \
             phase_ctx("p2"):
            for qo, qw in QBLKS:
                qsl = slice(qo, qo + qw)
                pv = ps_pv.tile([64 + VW, 512], F32, tag="pvacc")
                pvout = [pv[0:VW, 0:qw], pv[64 : 64 + VW, 0:qw]]

                def emit_s_exp(g):
                    units = [g * grp + j for j in range(grp) if g * grp + j < NU]
                    nu = len(units)
                    s = ps_s.tile([128, grp * 512], F32, tag="s")
                    for j, u in enumerate(units):
                        k, h = divmod(u, 2)
                        nc.tensor.matmul(
                            s[:, j * 512 : j * 512 + qw],
                            kT[_ts(h, DH), _ts(k, 128)],
                            qT[_ts(h, DH), qsl],
                            start=True, stop=True,
                        )
                    p = ptp.tile([128, grp * 512], BF16, tag="p")
                    nc.scalar.activation(
                        p[:].rearrange("q (j w) -> q j w", w=512)[:, 0:nu, 0:qw],
                        s[:].rearrange("q (j w) -> q j w", w=512)[:, 0:nu, 0:qw],
                        AF.Exp,
                    )
                    return units, p

                def emit_pv(units, p):
                    for j, u in enumerate(units):
                        k, h = divmod(u, 2)
                        base = k * 2 * VW + h * VW
                        nc.tensor.matmul(
                            pvout[h],
                            vaug[:, base : base + VW],
                            p[:, j * 512 : j * 512 + qw],
                            start=(k == 0), stop=(k == NKC - 1),
                        )

                # software pipeline: keep S(g+1) ahead of PV(g) in the PE
                # stream so the PE streams S while ACT runs exp(g), instead
                # of stalling in-order on the exp -> PV dependency.
                ngr = (NU + grp - 1) // grp
                pend = [emit_s_exp(0), emit_s_exp(1)]
                for g in range(2, ngr):
                    pend.append(emit_s_exp(g))
                    emit_pv(*pend.pop(0))
                while pend:
                    emit_pv(*pend.pop(0))
                nc.vector.tensor_copy(oT[0:DH, qsl], pv[0:DH, 0:qw])
                nc.vector.tensor_copy(dsum[0:1, qsl], pv[DH : DH + 1, 0:qw])
                nc.vector.tensor_copy(oT[DH : 2 * DH, qsl], pv[64 : 64 + DH, 0:qw])
                nc.vector.tensor_copy(dsum[32:33, qsl], pv[96:97, 0:qw])
                # 1/denominator for this q-block, overlapped with later blocks
                # (~51-ULP approx; denominators are ~3136 so edge cases moot)
                nc.vector.reciprocal_approx_fast(rdsum[:, qsl], dsum[:, qsl])
                nc.vector.tensor_copy(rdsb[:, qsl], rdsum[:, qsl])

        # ========== phase 3+4: normalize + output projection (Z^T) ==========
        ind33b = persist.tile([33, C], BF16)
        nc.vector.tensor_copy(ind33b[:], ind33[:])
        zT = persist.tile([C, N], F32)
        with tc.tile_pool(name="ps_fin", bufs=3, space="PSUM") as ps_fin, \
             tc.tile_pool(name="otn", bufs=3) as otnp, \
             phase_ctx("p34"):
            def emit_bcast(qo, qw):
                sl = slice(qo, qo + qw)
                bc = ps_fin.tile([C, 512], F32, tag="bc")
                nc.tensor.matmul(
                    bc[:, 0:qw], ind33b[:], rdsb[:, sl], start=True, stop=True
                )
                return bc

            def emit_out(bc, qo, qw):
                sl = slice(qo, qo + qw)
                otn = otnp.tile([INNER, 512], BF16, tag="otn")
                nc.vector.tensor_mul(otn[:, 0:qw], oT[:, sl], bc[:, 0:qw])
                zp = ps_fin.tile([C, 512], F32, tag="zp")
                nc.tensor.matmul(
                    zp[:, 0:qw], wo[:], otn[:, 0:qw], start=True, stop=True
                )
                nc.scalar.activation(
                    zT[:, sl], zp[:, 0:qw], AF.Identity, bias=boT[:]
                )

            # software pipeline: bcast(i+1) ahead of the dependent chain of i
            # so the PE never stalls in-order on the DVE normalize multiply.
            prev_fin = None
            for qo, qw in QBLKS:
                bc = emit_bcast(qo, qw)
                if prev_fin is not None:
                    emit_out(*prev_fin)
                prev_fin = (bc, qo, qw)
            emit_out(*prev_fin)
            nc.sync.dma_start(y_d[:], zT[:])

    nc.compile()
    return nc


_CACHED = {}


def _get_kernel():
    if "nc" not in _CACHED:
        _CACHED["nc"] = build_kernel()
    return _CACHED["nc"]


LAST_PERF = {}


def make_in_maps(x, context, Wq, Wk, Wv, Wo, bo):
    import ml_dtypes
    BF = ml_dtypes.bfloat16
    x = np.asarray(x, dtype=np.float32).astype(BF)
    context = np.asarray(context, dtype=np.float32).astype(BF)
    wq = np.ascontiguousarray(
        (np.asarray(Wq, dtype=np.float32) * np.float32(SCALE)).astype(BF))
    wk = np.ascontiguousarray(np.asarray(Wk, dtype=np.float32).astype(BF))
    wv = np.ascontiguousarray(np.asarray(Wv, dtype=np.float32).astype(BF))
    wo = np.ascontiguousarray(np.asarray(Wo, dtype=np.float32).astype(BF))
    boT = np.ascontiguousarray(np.asarray(bo, dtype=np.float32).reshape(C, 1))
    B = x.shape[0]
    in_maps = []
    for b in range(B):
        cTp = np.zeros((C, NKP), BF)
        cTp[:, :NK] = context[b].T
        in_maps.append(
            {
                "xT": np.ascontiguousarray(x[b].T),
                "cT": cTp,
                "wq": wq, "wk": wk, "wv": wv, "wo": wo, "bo": boT,
            }
        )
    return in_maps


def kernel(x, context, Wq, Wk, Wv, Wo, bo, _trace=False):
    in_maps = make_in_maps(x, context, Wq, Wk, Wv, Wo, bo)
    nc = _get_kernel()
    B = len(in_maps)
    res = bass_utils.run_bass_kernel_spmd(
        nc, in_maps, core_ids=list(range(B)), trace=_trace
    )
    LAST_PERF["exec_time_ns"] = res.exec_time_ns
    LAST_PERF["trace"] = res.instructions_and_trace
    # y is Z^T [64, 3136] per core; transpose back while unsharding
    out = np.stack(
        [np.ascontiguousarray(res.results[b]["y"].T) for b in range(B)], axis=0
    )
    return out

